# revision 24
# baseline (speedup 1.0000x reference)
"""Harmonic decomposition kernel for 8 TRN2 NeuronCores.

out[b] = basis^T R(theta_b) (basis @ x_b)   with per-harmonic complex rotation.

Sharding: the N*L*2 = 800 coefficient axis is split into 8 slices of 100 in
natural order (real parts on even rows, imaginary on odd rows). Each core
ships its basis slice ONCE in c-major layout (bc [100, 25600] f16), derives
the ij-major tiles needed by the projection on-chip (PE transpose -> PSUM f16
-> DVE/ACT evacuation), and runs both GEMMs with the basis slice as the
*stationary* matmul operand so each matmul only streams 8 batch columns:

  GEMM1 (projection):     coefT[c, b] += bijT_t[ij, c]^T @ xt_t[ij, b]
  rotate:                 rotT = coefT*ca + shuffle(coefT)*sa  (partition-pair
                          swap via stream_shuffle mask i^1; sa sign-folded)
  GEMM2 (reconstruction): outT_j[ij, b] = bc_j[c, ij]^T @ rotT[c, b]

A tunable tail of H ij-tiles is shipped pre-transposed from the host (bijh)
to balance PE-transpose cycles against DMA bandwidth and shorten the tail.
GEMM2 results are DMA'd straight from PSUM as f32 partials (no staging hop);
the host sums the 8 partials.
"""

import sys

import numpy as np

for _p in ("/opt/trn_rl_repo",):
    if _p not in sys.path:
        sys.path.insert(0, _p)

B = 8          # batch
IJ = 25600     # 160*160 spatial
NL2 = 800     # total coefficients
PC = 100       # coefficients per core
P = 128        # partitions
NT = 200       # ij tiles (IJ / 128)
NCORES = 8

# --- tunables ---------------------------------------------------------------
H = 52                                  # hybrid tiles shipped pre-transposed
TR = NT - H                             # tiles transposed on-chip
CH_TR = [4] + [20] * 7 + [4]            # bc chunk sizes over t < TR (sum 148)
CH_G2 = [26, 18, 8]                     # bc chunk sizes over t >= TR
STAGES = [4] + [10] * 14 + [4]          # transpose stages (chunk-aligned)
G1_LAG = 2                              # stages between transpose and G1 use
G1H_AT = 14                             # stage index after which G1-H is issued
CASA_AT = 1                             # stage index after which casa DMA rides ACT
XT_SPLIT = 40                           # tiles in the early xt slice
XT_B_AFTER = 3                          # chunk index after which xt_b ships
NSTG = 5                                # SBUF stage buffers
NPT = 3                                 # PSUM transpose buffers
GSZ = 64                                # GEMM2 j-tiles per PSUM bank
NPO = 4                                 # GEMM2 PSUM banks
assert sum(CH_TR) == TR and sum(CH_G2) == H and sum(STAGES) == TR


def _patch_tile_drain():
    """This container's walrus caps sem-waits at 1 per instruction; the stock
    Tile tail drain carries one wait per live semaphore. Keep one on the drain
    and emit the rest as individual SP wait instructions before the barrier."""
    import concourse.tile as tile
    from concourse.vector_clock import ScopedClock

    if getattr(tile.TileContext, "_ant_drain_patched", False):
        return

    def _drain_and_barrier(self, tick_clock, wait_clock):
        nc = self.nc
        drain_inst = nc.sync.drain()
        wait_clock.add_sem_waits(
            drain_inst.ins, ScopedClock({None: tick_clock.global_clock})
        )
        si = drain_inst.ins.sync_info
        waits = list(si.on_wait) if si and si.on_wait else []
        if len(waits) > 1:
            num2sem = {s.num: s for s in self.sems.allocated().values()}
            si.on_wait = waits[:1]
            for w in waits[1:]:
                op = {"sem-ge-imm": "sem-ge", "sem-eq-imm": "sem-eq"}[w.wait_mode]
                nc.sync.nop(nofuse=True).wait_op(num2sem[w.id], w.wait_value, op)
        nc.all_engine_barrier()
        assert self.sems is not None
        popped = nc._tile_sem_poison_stack.pop()
        assert popped is self._sem_poison
        nc.clear_and_free_semaphores(list(self.sems.allocated().values()))
        nc.all_engine_barrier()

    tile.TileContext._drain_and_barrier = _drain_and_barrier
    tile.TileContext._ant_drain_patched = True


def _split_excess_waits(nc, mybir):
    """Walrus in this container accepts at most 1 sem-wait per instruction
    (2 for EventSemaphore). Tile can attach several. Move the extras onto
    fresh NoOps inserted just before the instruction on the same engine —
    same-engine streams execute in order, so semantics are preserved."""
    ctr = 0
    for fn in nc.m.functions:
        for bb in fn.blocks:
            out, changed = [], False
            for inst in bb.instructions:
                si = inst.sync_info
                waits = list(si.on_wait) if si and si.on_wait else []
                cap = 2 if isinstance(inst, mybir.InstEventSemaphore) else 1
                if len(waits) > cap:
                    for w in waits[:-cap]:
                        nop = mybir.InstNoOp(name=f"I-wsplit-{ctr}", ins=[], outs=[])
                        ctr += 1
                        nop.engine = inst.engine
                        nop.sync_info = mybir.SyncInfo(on_wait=[w], on_update=[])
                        out.append(nop)
                    si.on_wait = waits[-cap:]
                    changed = True
                out.append(inst)
            if changed:
                bb.instructions = out


def _build():
    import concourse.bass as bass
    import concourse.mybir as mybir
    import concourse.tile as tile
    from concourse.masks import make_identity

    _patch_tile_drain()
    f16 = mybir.dt.float16
    f32 = mybir.dt.float32

    nc = bass.Bass()
    bc = nc.declare_dram_parameter("bc", [PC, IJ], f16, isOutput=False)
    xt = nc.declare_dram_parameter("xt", [P, NT * B], f16, isOutput=False)
    bijh = nc.declare_dram_parameter("bijh", [P, max(H, 1) * PC], f16, isOutput=False)
    casa = nc.declare_dram_parameter("casa", [PC, 2 * B], f32, isOutput=False)
    # outT tiles packed [ij_local 128, tile, b]: col 8*j + b = out_b[128*j + p]
    out = nc.declare_dram_parameter("out", [P, NT * B], f16, isOutput=True)

    with tile.TileContext(nc) as tc:
        with (
            tc.tile_pool(name="big", bufs=1) as big,
            tc.tile_pool(name="small", bufs=1) as small,
            tc.tile_pool(name="stg", bufs=NSTG) as stg,
            tc.tile_pool(name="pt", bufs=NPT, space="PSUM") as ptp,
            tc.tile_pool(name="co", bufs=1, space="PSUM") as cop,
            tc.tile_pool(name="po", bufs=NPO, space="PSUM") as pop,
        ):
            BC = big.tile([PC, IJ], f16, tag="bc")
            XT = big.tile([P, NT * B], f16, tag="xt")
            BIJH = big.tile([P, max(H, 1) * PC], f16, tag="bijh")
            CASA = small.tile([PC, 2 * B], f32, tag="casa")
            IDENT = small.tile([PC, PC], f16, tag="ident")

            # identity for PE transpose, built on the otherwise-idle gpsimd
            make_identity(nc, IDENT[:])

            # Input DMAs, FIFO on SP's ring: bc chunks with an early xt slice
            # (G1 consumes xt tile-by-tile) and the rest of xt a few chunks
            # later; bijh then the G2-only bc range last, so the final arrival
            # gates only GEMM2's smallest bank. casa rides the ACT ring
            # mid-stream (see stage loop) to keep HWDGE clear at startup.
            t0 = 0
            for i, cn in enumerate(CH_TR):
                nc.sync.dma_start(
                    BC[:, t0 * P : (t0 + cn) * P], bc[:, t0 * P : (t0 + cn) * P]
                )
                t0 += cn
                if i == 0:
                    nc.sync.dma_start(
                        XT[:, 0 : XT_SPLIT * B], xt[:, 0 : XT_SPLIT * B]
                    )
                if i + 1 == XT_B_AFTER:
                    nc.sync.dma_start(
                        XT[:, XT_SPLIT * B :], xt[:, XT_SPLIT * B :]
                    )
            if H > 0:
                nc.sync.dma_start(BIJH[:], bijh[:])
            for cn in CH_G2:
                nc.sync.dma_start(BC[:, t0 * P : (t0 + cn) * P], bc[:, t0 * P : (t0 + cn) * P])
                t0 += cn

            # coefT accumulator [128, 8] f32; zero the pad quadrant before the
            # G1 group opens (start=True re-resets rows 0:100, leaving 100:128
            # zero for the partition shuffle below)
            CO = cop.tile([P, B], f32, tag="co")
            nc.vector.memset(CO[96:P, :], 0.0)

            # GEMM1 pipeline: PE transposes stage s into PSUM (f16), DVE/ACT
            # evacuate to an SBUF stage buffer, PE consumes the stage as
            # *stationary* weights (8-col matmuls) G1_LAG stages behind the
            # transposes so the PE->evac->PE round trip stays off the PE's
            # critical path.
            stages = []
            t0 = 0
            for n in STAGES:
                stages.append((t0, n))
                t0 += n
            stage_tiles = []   # (STG tile, stage) ready for G1
            g1_done = 0

            def g1_consume():
                nonlocal g1_done
                ST, (ct, cn) = stage_tiles.pop(0)
                for k in range(cn):
                    t = ct + k
                    nc.tensor.matmul(
                        CO[0:PC, :],
                        ST[:, k * PC : (k + 1) * PC],
                        XT[:, t * B : (t + 1) * B],
                        start=(g1_done == 0),
                        stop=(g1_done == NT - 1),
                    )
                    g1_done += 1

            def g1_hybrid():
                # hybrid pre-transposed tiles, issued mid-stream (once bijh has
                # landed) so they stay off the tail critical path
                nonlocal g1_done
                for i in range(H):
                    t = TR + i
                    nc.tensor.matmul(
                        CO[0:PC, :],
                        BIJH[:, i * PC : (i + 1) * PC],
                        XT[:, t * B : (t + 1) * B],
                        start=(g1_done == 0),
                        stop=(g1_done == NT - 1),
                    )
                    g1_done += 1

            for s, (ct, cn) in enumerate(stages):
                PT = ptp.tile([P, 10 * PC], f16, tag="pt")
                for k in range(cn):
                    nc.tensor.transpose(
                        PT[:, k * PC : (k + 1) * PC],
                        BC[:, (ct + k) * P : (ct + k + 1) * P],
                        IDENT[:],
                    )
                ST = stg.tile([P, 10 * PC], f16, tag="stg")
                if s % 2 == 1:
                    nc.scalar.copy(ST[:, 0 : cn * PC], PT[:, 0 : cn * PC])
                else:
                    nc.vector.tensor_copy(ST[:, 0 : cn * PC], PT[:, 0 : cn * PC])
                stage_tiles.append((ST, (ct, cn)))
                if s == CASA_AT:
                    nc.scalar.dma_start(CASA[:], casa[:])
                if len(stage_tiles) > G1_LAG:
                    g1_consume()
                if s == G1H_AT and H > 0:
                    g1_hybrid()
            while stage_tiles:
                g1_consume()
            if G1H_AT >= len(stages) and H > 0:
                g1_hybrid()
            assert g1_done == NT

            # rotation: rotT = coefT*ca + swap_pairs(coefT)*sa (sa sign-folded)
            CSW = small.tile([P, B], f32, tag="csw")
            mask = [i ^ 1 for i in range(32)]
            nc.vector.stream_shuffle(CSW[:], CO[:], mask)
            TMP = small.tile([PC, B], f32, tag="tmp")
            TMP2 = small.tile([PC, B], f32, tag="tmp2")
            ROTT = small.tile([PC, B], f16, tag="rott")
            nc.vector.tensor_mul(TMP[:], CO[0:PC, :], CASA[:, 0:B])
            nc.vector.tensor_mul(TMP2[:], CSW[0:PC, :], CASA[:, B : 2 * B])
            nc.vector.tensor_add(ROTT[:], TMP[:], TMP2[:])

            # GEMM2: outT_j = bc_j^T @ rotT, 64 j-tiles per PSUM bank.
            # Bank evacs alternate ACT/DVE; paired output DMAs ride both
            # HWDGE rings, ordered so no DMA blocks a later evac on its ring.
            OST = big.tile([P, NT * B], f16, tag="ost")
            ngroups = (NT + GSZ - 1) // GSZ
            bank_w = []
            for g in range(ngroups):
                js = range(g * GSZ, min((g + 1) * GSZ, NT))
                PO = pop.tile([P, GSZ * B], f32, tag="po")
                for i, j in enumerate(js):
                    nc.tensor.matmul(
                        PO[:, i * B : (i + 1) * B],
                        BC[:, j * P : (j + 1) * P],
                        ROTT[:],
                        start=True,
                        stop=True,
                    )
                w = len(js) * B
                o0 = g * GSZ * B
                if g % 2 == 0:
                    nc.scalar.copy(OST[:, o0 : o0 + w], PO[:, 0:w])
                else:
                    nc.vector.tensor_copy(OST[:, o0 : o0 + w], PO[:, 0:w])
                bank_w.append(w)
                if g == 1:
                    nc.sync.dma_start(
                        out[:, 0 : bank_w[0] + bank_w[1]],
                        OST[:, 0 : bank_w[0] + bank_w[1]],
                    )
                elif g == 2:
                    o0 = bank_w[0] + bank_w[1]
                    nc.scalar.dma_start(out[:, o0 : o0 + w], OST[:, o0 : o0 + w])
            o0 = sum(bank_w[:3])
            w = sum(bank_w[3:])
            nc.sync.dma_start(out[:, o0 : o0 + w], OST[:, o0 : o0 + w])
    _split_excess_waits(nc, mybir)
    return nc


_CACHE = {}


def _get_nc():
    if "nc" not in _CACHE:
        _CACHE["nc"] = _build()
    return _CACHE["nc"]


def prep_in_maps(x, basis, angles):
    x = np.asarray(x)
    basis = np.asarray(basis)
    angles = np.asarray(angles).astype(np.float32)

    X2 = x.reshape(B, IJ)
    BF = basis.reshape(NL2, IJ)
    xt16 = np.ascontiguousarray(
        X2.T.reshape(NT, P, B).transpose(1, 0, 2)
    ).reshape(P, NT * B).astype(np.float16)

    j = np.arange(PC)
    sign = np.where(j % 2 == 0, 1.0, -1.0).astype(np.float32)

    in_maps = []
    for k in range(NCORES):
        bc16 = BF[k * PC : (k + 1) * PC].astype(np.float16)     # natural order
        if H > 0:
            bijh = np.ascontiguousarray(
                bc16[:, TR * P :].reshape(PC, H, P).transpose(2, 1, 0)
            ).reshape(P, H * PC)
        else:
            bijh = np.zeros((P, PC), dtype=np.float16)
        lvals = ((k * 50 + j // 2) % 20).astype(np.float32)
        theta = lvals[:, None] * angles[None, :]                # [100, 8]
        casa = np.concatenate(
            [np.cos(theta), np.sin(theta) * sign[:, None]], axis=1
        ).astype(np.float32)                                    # [100, 16]
        in_maps.append({"bc": bc16, "xt": xt16, "bijh": bijh, "casa": casa})
    return in_maps


def kernel(x, basis, angles):
    from concourse.bass_utils import run_bass_kernel_spmd

    in_maps = prep_in_maps(x, basis, angles)
    res = run_bass_kernel_spmd(_get_nc(), in_maps, list(range(NCORES)))
    stage = np.zeros((P, NT * B), dtype=np.float32)
    for k in range(NCORES):
        stage += res.results[k]["out"].astype(np.float32)
    # col 8*j + b, row p  ->  out[b, 128*j + p]
    total = stage.reshape(P, NT, B).transpose(2, 1, 0).reshape(B, IJ)
    return np.ascontiguousarray(total).reshape(B, 1, 160, 160)


# revision 27
# speedup vs baseline: 1.0177x; 1.0177x over previous
"""Harmonic decomposition kernel for 8 TRN2 NeuronCores.

out[b] = basis^T R(theta_b) (basis @ x_b)   with per-harmonic complex rotation.

Sharding: the N*L*2 = 800 coefficient axis is split into 8 slices of 100 in
natural order (real parts on even rows, imaginary on odd rows). Each core
ships its basis slice ONCE in c-major layout (bc [100, 25600] f16), derives
the ij-major tiles needed by the projection on-chip (PE transpose -> PSUM f16
-> DVE/ACT evacuation), and runs both GEMMs with the basis slice as the
*stationary* matmul operand so each matmul only streams 8 batch columns:

  GEMM1 (projection):     coefT[c, b] += bijT_t[ij, c]^T @ xt_t[ij, b]
  rotate:                 rotT = coefT*ca + shuffle(coefT)*sa  (partition-pair
                          swap via stream_shuffle mask i^1; sa sign-folded)
  GEMM2 (reconstruction): outT_j[ij, b] = bc_j[c, ij]^T @ rotT[c, b]

A tunable tail of H ij-tiles is shipped pre-transposed from the host (bijh)
to balance PE-transpose cycles against DMA bandwidth and shorten the tail.
GEMM2 results are DMA'd straight from PSUM as f32 partials (no staging hop);
the host sums the 8 partials.
"""

import sys

import numpy as np

for _p in ("/opt/trn_rl_repo",):
    if _p not in sys.path:
        sys.path.insert(0, _p)

B = 8          # batch
IJ = 25600     # 160*160 spatial
NL2 = 800     # total coefficients
PC = 100       # coefficients per core
P = 128        # partitions
NT = 200       # ij tiles (IJ / 128)
NCORES = 8

# --- tunables ---------------------------------------------------------------
H = 52                                  # hybrid tiles shipped pre-transposed
TR = NT - H                             # tiles transposed on-chip
CH_TR = [4] + [20] * 7 + [4]            # bc chunk sizes over t < TR (sum 148)
CH_G2 = [26, 18, 8]                     # bc chunk sizes over t >= TR
STAGES = [4] + [10] * 14 + [4]          # transpose stages (chunk-aligned)
G1_LAG = 2                              # stages between transpose and G1 use
G1H_AT = 14                             # stage index after which G1-H is issued
CASA_AT = 5                             # stage index after which casa DMA rides ACT
XT_SPLIT = 40                           # tiles in the early xt slice
XT_A_AFTER = 2                          # chunk index after which xt_a ships
XT_B_AFTER = 4                          # chunk index after which xt_b ships
NSTG = 5                                # SBUF stage buffers
NPT = 3                                 # PSUM transpose buffers
GSZ = 64                                # GEMM2 j-tiles per PSUM bank
NPO = 4                                 # GEMM2 PSUM banks
assert sum(CH_TR) == TR and sum(CH_G2) == H and sum(STAGES) == TR


def _patch_tile_drain():
    """This container's walrus caps sem-waits at 1 per instruction; the stock
    Tile tail drain carries one wait per live semaphore. Keep one on the drain
    and emit the rest as individual SP wait instructions before the barrier."""
    import concourse.tile as tile
    from concourse.vector_clock import ScopedClock

    if getattr(tile.TileContext, "_ant_drain_patched", False):
        return

    def _drain_and_barrier(self, tick_clock, wait_clock):
        nc = self.nc
        drain_inst = nc.sync.drain()
        wait_clock.add_sem_waits(
            drain_inst.ins, ScopedClock({None: tick_clock.global_clock})
        )
        si = drain_inst.ins.sync_info
        waits = list(si.on_wait) if si and si.on_wait else []
        if len(waits) > 1:
            num2sem = {s.num: s for s in self.sems.allocated().values()}
            si.on_wait = waits[:1]
            for w in waits[1:]:
                op = {"sem-ge-imm": "sem-ge", "sem-eq-imm": "sem-eq"}[w.wait_mode]
                nc.sync.nop(nofuse=True).wait_op(num2sem[w.id], w.wait_value, op)
        nc.all_engine_barrier()
        assert self.sems is not None
        popped = nc._tile_sem_poison_stack.pop()
        assert popped is self._sem_poison
        nc.clear_and_free_semaphores(list(self.sems.allocated().values()))
        nc.all_engine_barrier()

    tile.TileContext._drain_and_barrier = _drain_and_barrier
    tile.TileContext._ant_drain_patched = True


def _split_excess_waits(nc, mybir):
    """Walrus in this container accepts at most 1 sem-wait per instruction
    (2 for EventSemaphore). Tile can attach several. Move the extras onto
    fresh NoOps inserted just before the instruction on the same engine —
    same-engine streams execute in order, so semantics are preserved."""
    ctr = 0
    for fn in nc.m.functions:
        for bb in fn.blocks:
            out, changed = [], False
            for inst in bb.instructions:
                si = inst.sync_info
                waits = list(si.on_wait) if si and si.on_wait else []
                cap = 2 if isinstance(inst, mybir.InstEventSemaphore) else 1
                if len(waits) > cap:
                    for w in waits[:-cap]:
                        nop = mybir.InstNoOp(name=f"I-wsplit-{ctr}", ins=[], outs=[])
                        ctr += 1
                        nop.engine = inst.engine
                        nop.sync_info = mybir.SyncInfo(on_wait=[w], on_update=[])
                        out.append(nop)
                    si.on_wait = waits[-cap:]
                    changed = True
                out.append(inst)
            if changed:
                bb.instructions = out


def _build():
    import concourse.bass as bass
    import concourse.mybir as mybir
    import concourse.tile as tile
    from concourse.masks import make_identity

    _patch_tile_drain()
    f16 = mybir.dt.float16
    f32 = mybir.dt.float32

    nc = bass.Bass()
    bc = nc.declare_dram_parameter("bc", [PC, IJ], f16, isOutput=False)
    xt = nc.declare_dram_parameter("xt", [P, NT * B], f16, isOutput=False)
    bijh = nc.declare_dram_parameter("bijh", [P, max(H, 1) * PC], f16, isOutput=False)
    casa = nc.declare_dram_parameter("casa", [PC, 2 * B], f32, isOutput=False)
    # outT tiles packed [ij_local 128, tile, b]: col 8*j + b = out_b[128*j + p]
    out = nc.declare_dram_parameter("out", [P, NT * B], f16, isOutput=True)

    with tile.TileContext(nc) as tc:
        with (
            tc.tile_pool(name="big", bufs=1) as big,
            tc.tile_pool(name="small", bufs=1) as small,
            tc.tile_pool(name="stg", bufs=NSTG) as stg,
            tc.tile_pool(name="pt", bufs=NPT, space="PSUM") as ptp,
            tc.tile_pool(name="co", bufs=1, space="PSUM") as cop,
            tc.tile_pool(name="po", bufs=NPO, space="PSUM") as pop,
        ):
            BC = big.tile([PC, IJ], f16, tag="bc")
            XT = big.tile([P, NT * B], f16, tag="xt")
            BIJH = big.tile([P, max(H, 1) * PC], f16, tag="bijh")
            CASA = small.tile([PC, 2 * B], f32, tag="casa")
            IDENT = small.tile([PC, PC], f16, tag="ident")

            # identity for PE transpose, built on the otherwise-idle gpsimd
            make_identity(nc, IDENT[:])

            # Input DMAs, FIFO on SP's ring: bc chunks with an early xt slice
            # (G1 consumes xt tile-by-tile) and the rest of xt a few chunks
            # later; bijh then the G2-only bc range last, so the final arrival
            # gates only GEMM2's smallest bank. casa rides the ACT ring
            # mid-stream (see stage loop) to keep HWDGE clear at startup.
            t0 = 0
            for i, cn in enumerate(CH_TR):
                nc.sync.dma_start(
                    BC[:, t0 * P : (t0 + cn) * P], bc[:, t0 * P : (t0 + cn) * P]
                )
                t0 += cn
                if i + 1 == XT_A_AFTER:
                    nc.sync.dma_start(
                        XT[:, 0 : XT_SPLIT * B], xt[:, 0 : XT_SPLIT * B]
                    )
                if i + 1 == XT_B_AFTER:
                    nc.sync.dma_start(
                        XT[:, XT_SPLIT * B :], xt[:, XT_SPLIT * B :]
                    )
            if H > 0:
                nc.sync.dma_start(BIJH[:], bijh[:])
            for cn in CH_G2:
                nc.sync.dma_start(BC[:, t0 * P : (t0 + cn) * P], bc[:, t0 * P : (t0 + cn) * P])
                t0 += cn

            # coefT accumulator [128, 8] f32; zero the pad quadrant before the
            # G1 group opens (start=True re-resets rows 0:100, leaving 100:128
            # zero for the partition shuffle below)
            CO = cop.tile([P, B], f32, tag="co")
            nc.vector.memset(CO[96:P, :], 0.0)

            # GEMM1 pipeline: PE transposes stage s into PSUM (f16), DVE/ACT
            # evacuate to an SBUF stage buffer, PE consumes the stage as
            # *stationary* weights (8-col matmuls) G1_LAG stages behind the
            # transposes so the PE->evac->PE round trip stays off the PE's
            # critical path.
            stages = []
            t0 = 0
            for n in STAGES:
                stages.append((t0, n))
                t0 += n
            stage_tiles = []   # (STG tile, stage) ready for G1
            g1_done = 0

            def g1_consume():
                nonlocal g1_done
                ST, (ct, cn) = stage_tiles.pop(0)
                for k in range(cn):
                    t = ct + k
                    nc.tensor.matmul(
                        CO[0:PC, :],
                        ST[:, k * PC : (k + 1) * PC],
                        XT[:, t * B : (t + 1) * B],
                        start=(g1_done == 0),
                        stop=(g1_done == NT - 1),
                    )
                    g1_done += 1

            def g1_hybrid():
                # hybrid pre-transposed tiles, issued mid-stream (once bijh has
                # landed) so they stay off the tail critical path
                nonlocal g1_done
                for i in range(H):
                    t = TR + i
                    nc.tensor.matmul(
                        CO[0:PC, :],
                        BIJH[:, i * PC : (i + 1) * PC],
                        XT[:, t * B : (t + 1) * B],
                        start=(g1_done == 0),
                        stop=(g1_done == NT - 1),
                    )
                    g1_done += 1

            for s, (ct, cn) in enumerate(stages):
                PT = ptp.tile([P, 10 * PC], f16, tag="pt")
                for k in range(cn):
                    nc.tensor.transpose(
                        PT[:, k * PC : (k + 1) * PC],
                        BC[:, (ct + k) * P : (ct + k + 1) * P],
                        IDENT[:],
                    )
                ST = stg.tile([P, 10 * PC], f16, tag="stg")
                if s % 2 == 1:
                    nc.scalar.copy(ST[:, 0 : cn * PC], PT[:, 0 : cn * PC])
                else:
                    nc.vector.tensor_copy(ST[:, 0 : cn * PC], PT[:, 0 : cn * PC])
                stage_tiles.append((ST, (ct, cn)))
                if s == CASA_AT:
                    nc.scalar.dma_start(CASA[:], casa[:])
                if len(stage_tiles) > G1_LAG:
                    g1_consume()
                if s == G1H_AT and H > 0:
                    g1_hybrid()
            while stage_tiles:
                g1_consume()
            if G1H_AT >= len(stages) and H > 0:
                g1_hybrid()
            assert g1_done == NT

            # rotation: rotT = coefT*ca + swap_pairs(coefT)*sa (sa sign-folded)
            CSW = small.tile([P, B], f32, tag="csw")
            mask = [i ^ 1 for i in range(32)]
            nc.vector.stream_shuffle(CSW[:], CO[:], mask)
            TMP = small.tile([PC, B], f32, tag="tmp")
            TMP2 = small.tile([PC, B], f32, tag="tmp2")
            ROTT = small.tile([PC, B], f16, tag="rott")
            nc.vector.tensor_mul(TMP[:], CO[0:PC, :], CASA[:, 0:B])
            nc.vector.tensor_mul(TMP2[:], CSW[0:PC, :], CASA[:, B : 2 * B])
            nc.vector.tensor_add(ROTT[:], TMP[:], TMP2[:])

            # GEMM2: outT_j = bc_j^T @ rotT, 64 j-tiles per PSUM bank.
            # Bank evacs alternate ACT/DVE; paired output DMAs ride both
            # HWDGE rings, ordered so no DMA blocks a later evac on its ring.
            OST = big.tile([P, NT * B], f16, tag="ost")
            ngroups = (NT + GSZ - 1) // GSZ
            bank_w = []
            for g in range(ngroups):
                js = range(g * GSZ, min((g + 1) * GSZ, NT))
                PO = pop.tile([P, GSZ * B], f32, tag="po")
                for i, j in enumerate(js):
                    nc.tensor.matmul(
                        PO[:, i * B : (i + 1) * B],
                        BC[:, j * P : (j + 1) * P],
                        ROTT[:],
                        start=True,
                        stop=True,
                    )
                w = len(js) * B
                o0 = g * GSZ * B
                if g % 2 == 0:
                    nc.scalar.copy(OST[:, o0 : o0 + w], PO[:, 0:w])
                else:
                    nc.vector.tensor_copy(OST[:, o0 : o0 + w], PO[:, 0:w])
                bank_w.append(w)
                if g == 1:
                    nc.sync.dma_start(
                        out[:, 0 : bank_w[0] + bank_w[1]],
                        OST[:, 0 : bank_w[0] + bank_w[1]],
                    )
            o0 = bank_w[0] + bank_w[1]
            w = sum(bank_w[2:])
            nc.scalar.dma_start(out[:, o0 : o0 + w], OST[:, o0 : o0 + w])
    _split_excess_waits(nc, mybir)
    return nc


_CACHE = {}


def _get_nc():
    if "nc" not in _CACHE:
        _CACHE["nc"] = _build()
    return _CACHE["nc"]


def prep_in_maps(x, basis, angles):
    x = np.asarray(x)
    basis = np.asarray(basis)
    angles = np.asarray(angles).astype(np.float32)

    X2 = x.reshape(B, IJ)
    BF = basis.reshape(NL2, IJ)
    xt16 = np.ascontiguousarray(
        X2.T.reshape(NT, P, B).transpose(1, 0, 2)
    ).reshape(P, NT * B).astype(np.float16)

    j = np.arange(PC)
    sign = np.where(j % 2 == 0, 1.0, -1.0).astype(np.float32)

    in_maps = []
    for k in range(NCORES):
        bc16 = BF[k * PC : (k + 1) * PC].astype(np.float16)     # natural order
        if H > 0:
            bijh = np.ascontiguousarray(
                bc16[:, TR * P :].reshape(PC, H, P).transpose(2, 1, 0)
            ).reshape(P, H * PC)
        else:
            bijh = np.zeros((P, PC), dtype=np.float16)
        lvals = ((k * 50 + j // 2) % 20).astype(np.float32)
        theta = lvals[:, None] * angles[None, :]                # [100, 8]
        casa = np.concatenate(
            [np.cos(theta), np.sin(theta) * sign[:, None]], axis=1
        ).astype(np.float32)                                    # [100, 16]
        in_maps.append({"bc": bc16, "xt": xt16, "bijh": bijh, "casa": casa})
    return in_maps


def kernel(x, basis, angles):
    from concourse.bass_utils import run_bass_kernel_spmd

    in_maps = prep_in_maps(x, basis, angles)
    res = run_bass_kernel_spmd(_get_nc(), in_maps, list(range(NCORES)))
    stage = np.zeros((P, NT * B), dtype=np.float32)
    for k in range(NCORES):
        stage += res.results[k]["out"].astype(np.float32)
    # col 8*j + b, row p  ->  out[b, 128*j + p]
    total = stage.reshape(P, NT, B).transpose(2, 1, 0).reshape(B, IJ)
    return np.ascontiguousarray(total).reshape(B, 1, 160, 160)


# revision 33
# speedup vs baseline: 1.0489x; 1.0307x over previous
"""Harmonic decomposition kernel for 8 TRN2 NeuronCores.

out[b] = basis^T R(theta_b) (basis @ x_b)   with per-harmonic complex rotation.

Sharding: the N*L*2 = 800 coefficient axis is split into 8 slices of 100 in
natural order (real parts on even rows, imaginary on odd rows). Each core
ships its basis slice ONCE in c-major layout (bc [100, 25600] f16), derives
the ij-major tiles needed by the projection on-chip (PE transpose -> PSUM f16
-> DVE/ACT evacuation), and runs both GEMMs with the basis slice as the
*stationary* matmul operand so each matmul only streams 8 batch columns:

  GEMM1 (projection):     coefT[c, b] += bijT_t[ij, c]^T @ xt_t[ij, b]
  rotate:                 rotT = coefT*ca + shuffle(coefT)*sa  (partition-pair
                          swap via stream_shuffle mask i^1; sa sign-folded)
  GEMM2 (reconstruction): outT_j[ij, b] = bc_j[c, ij]^T @ rotT[c, b]

A tunable tail of H ij-tiles is shipped pre-transposed from the host (bijh)
to balance PE-transpose cycles against DMA bandwidth and shorten the tail.
GEMM2 results are DMA'd straight from PSUM as f32 partials (no staging hop);
the host sums the 8 partials.
"""

import sys

import numpy as np

for _p in ("/opt/trn_rl_repo",):
    if _p not in sys.path:
        sys.path.insert(0, _p)

B = 8          # batch
IJ = 25600     # 160*160 spatial
NL2 = 800     # total coefficients
PC = 100       # coefficients per core
P = 128        # partitions
NT = 200       # ij tiles (IJ / 128)
NCORES = 8

# --- tunables ---------------------------------------------------------------
H = 52                                  # hybrid tiles shipped pre-transposed
TR = NT - H                             # tiles transposed on-chip
CH_TR = [8] + [20] * 6 + [16, 4]        # bc chunk sizes over t < TR (sum 148)
CH_G2 = [26, 18, 8]                     # bc chunk sizes over t >= TR
STAGES = [8] + [10] * 13 + [6, 4]       # transpose stages (chunk-aligned)
G1_LAG = 2                              # stages between transpose and G1 use
G1H_AT = 14                             # stage index after which G1-H is issued
XT_SPLIT = 40                           # tiles in the early xt slice
XT_A_AFTER = 2                          # chunk index after which xt_a ships
XT_B_AFTER = 4                          # chunk index after which xt_b ships
NSTG = 5                                # SBUF stage buffers
NPT = 3                                 # PSUM transpose buffers
GSZ = 64                                # GEMM2 j-tiles per PSUM bank
NPO = 4                                 # GEMM2 PSUM banks
assert sum(CH_TR) == TR and sum(CH_G2) == H and sum(STAGES) == TR


def _patch_tile_drain():
    """This container's walrus caps sem-waits at 1 per instruction; the stock
    Tile tail drain carries one wait per live semaphore. Keep one on the drain
    and emit the rest as individual SP wait instructions before the barrier."""
    import concourse.tile as tile
    from concourse.vector_clock import ScopedClock

    if getattr(tile.TileContext, "_ant_drain_patched", False):
        return

    def _drain_and_barrier(self, tick_clock, wait_clock):
        nc = self.nc
        drain_inst = nc.sync.drain()
        wait_clock.add_sem_waits(
            drain_inst.ins, ScopedClock({None: tick_clock.global_clock})
        )
        si = drain_inst.ins.sync_info
        waits = list(si.on_wait) if si and si.on_wait else []
        if len(waits) > 1:
            num2sem = {s.num: s for s in self.sems.allocated().values()}
            si.on_wait = waits[:1]
            for w in waits[1:]:
                op = {"sem-ge-imm": "sem-ge", "sem-eq-imm": "sem-eq"}[w.wait_mode]
                nc.sync.nop(nofuse=True).wait_op(num2sem[w.id], w.wait_value, op)
        nc.all_engine_barrier()
        assert self.sems is not None
        popped = nc._tile_sem_poison_stack.pop()
        assert popped is self._sem_poison
        nc.clear_and_free_semaphores(list(self.sems.allocated().values()))
        nc.all_engine_barrier()

    tile.TileContext._drain_and_barrier = _drain_and_barrier
    tile.TileContext._ant_drain_patched = True


def _split_excess_waits(nc, mybir):
    """Walrus in this container accepts at most 1 sem-wait per instruction
    (2 for EventSemaphore). Tile can attach several. Move the extras onto
    fresh NoOps inserted just before the instruction on the same engine —
    same-engine streams execute in order, so semantics are preserved."""
    ctr = 0
    for fn in nc.m.functions:
        for bb in fn.blocks:
            out, changed = [], False
            for inst in bb.instructions:
                si = inst.sync_info
                waits = list(si.on_wait) if si and si.on_wait else []
                cap = 2 if isinstance(inst, mybir.InstEventSemaphore) else 1
                if len(waits) > cap:
                    for w in waits[:-cap]:
                        nop = mybir.InstNoOp(name=f"I-wsplit-{ctr}", ins=[], outs=[])
                        ctr += 1
                        nop.engine = inst.engine
                        nop.sync_info = mybir.SyncInfo(on_wait=[w], on_update=[])
                        out.append(nop)
                    si.on_wait = waits[-cap:]
                    changed = True
                out.append(inst)
            if changed:
                bb.instructions = out


def _build():
    import concourse.bass as bass
    import concourse.mybir as mybir
    import concourse.tile as tile
    from concourse.masks import make_identity

    _patch_tile_drain()
    f16 = mybir.dt.float16
    f32 = mybir.dt.float32

    nc = bass.Bass()
    bc = nc.declare_dram_parameter("bc", [PC, IJ], f16, isOutput=False)
    xt = nc.declare_dram_parameter("xt", [P, NT * B], f16, isOutput=False)
    # bijh carries the hybrid pre-transposed tiles plus 32 trailing f16
    # columns holding the f32 rotation table (ca|sa) bitcast to f16 words
    bijh = nc.declare_dram_parameter(
        "bijh", [P, max(H, 1) * PC + 32], f16, isOutput=False
    )
    # outT tiles packed [ij_local 128, tile, b]: col 8*j + b = out_b[128*j + p]
    out = nc.declare_dram_parameter("out", [P, NT * B], f16, isOutput=True)

    with tile.TileContext(nc) as tc:
        with (
            tc.tile_pool(name="big", bufs=1) as big,
            tc.tile_pool(name="small", bufs=1) as small,
            tc.tile_pool(name="stg", bufs=NSTG) as stg,
            tc.tile_pool(name="pt", bufs=NPT, space="PSUM") as ptp,
            tc.tile_pool(name="co", bufs=1, space="PSUM") as cop,
            tc.tile_pool(name="po", bufs=NPO, space="PSUM") as pop,
        ):
            BC = big.tile([PC, IJ], f16, tag="bc")
            XT = big.tile([P, NT * B], f16, tag="xt")
            BIJH = big.tile([P, max(H, 1) * PC + 32], f16, tag="bijh")
            CASA = BIJH[0:PC, max(H, 1) * PC : max(H, 1) * PC + 32].bitcast(f32)
            IDENT = small.tile([PC, PC], f16, tag="ident")

            # identity for PE transpose, built on the otherwise-idle gpsimd
            make_identity(nc, IDENT[:])

            # Input DMAs, FIFO on SP's ring: bc chunks with an early xt slice
            # (G1 consumes xt tile-by-tile) and the rest of xt a few chunks
            # later; bijh (which carries the rotation table) then the G2-only
            # bc range last, so the final arrival gates only GEMM2's smallest
            # bank.
            t0 = 0
            for i, cn in enumerate(CH_TR):
                nc.sync.dma_start(
                    BC[:, t0 * P : (t0 + cn) * P], bc[:, t0 * P : (t0 + cn) * P]
                )
                t0 += cn
                if i + 1 == XT_A_AFTER:
                    nc.sync.dma_start(
                        XT[:, 0 : XT_SPLIT * B], xt[:, 0 : XT_SPLIT * B]
                    )
                if i + 1 == XT_B_AFTER:
                    nc.sync.dma_start(
                        XT[:, XT_SPLIT * B :], xt[:, XT_SPLIT * B :]
                    )
            if H > 0:
                nc.sync.dma_start(BIJH[:], bijh[:])
            for cn in CH_G2:
                nc.sync.dma_start(BC[:, t0 * P : (t0 + cn) * P], bc[:, t0 * P : (t0 + cn) * P])
                t0 += cn

            # coefT accumulator [128, 8] f32; zero the pad quadrant before the
            # G1 group opens (start=True re-resets rows 0:100, leaving 100:128
            # zero for the partition shuffle below)
            CO = cop.tile([P, B], f32, tag="co")
            nc.vector.memset(CO[96:P, :], 0.0)

            # GEMM1 pipeline: PE transposes stage s into PSUM (f16), DVE/ACT
            # evacuate to an SBUF stage buffer, PE consumes the stage as
            # *stationary* weights (8-col matmuls) G1_LAG stages behind the
            # transposes so the PE->evac->PE round trip stays off the PE's
            # critical path.
            stages = []
            t0 = 0
            for n in STAGES:
                stages.append((t0, n))
                t0 += n
            stage_tiles = []   # (STG tile, stage) ready for G1
            g1_done = 0

            def g1_consume():
                nonlocal g1_done
                ST, (ct, cn) = stage_tiles.pop(0)
                for k in range(cn):
                    t = ct + k
                    nc.tensor.matmul(
                        CO[0:PC, :],
                        ST[:, k * PC : (k + 1) * PC],
                        XT[:, t * B : (t + 1) * B],
                        start=(g1_done == 0),
                        stop=(g1_done == NT - 1),
                    )
                    g1_done += 1

            def g1_hybrid():
                # hybrid pre-transposed tiles, issued mid-stream (once bijh has
                # landed) so they stay off the tail critical path
                nonlocal g1_done
                for i in range(H):
                    t = TR + i
                    nc.tensor.matmul(
                        CO[0:PC, :],
                        BIJH[:, i * PC : (i + 1) * PC],
                        XT[:, t * B : (t + 1) * B],
                        start=(g1_done == 0),
                        stop=(g1_done == NT - 1),
                    )
                    g1_done += 1

            for s, (ct, cn) in enumerate(stages):
                PT = ptp.tile([P, 10 * PC], f16, tag="pt")
                for k in range(cn):
                    nc.tensor.transpose(
                        PT[:, k * PC : (k + 1) * PC],
                        BC[:, (ct + k) * P : (ct + k + 1) * P],
                        IDENT[:],
                    )
                ST = stg.tile([P, 10 * PC], f16, tag="stg")
                if s % 2 == 1:
                    nc.scalar.copy(ST[:, 0 : cn * PC], PT[:, 0 : cn * PC])
                else:
                    nc.vector.tensor_copy(ST[:, 0 : cn * PC], PT[:, 0 : cn * PC])
                stage_tiles.append((ST, (ct, cn)))
                if len(stage_tiles) > G1_LAG:
                    g1_consume()
                if s == G1H_AT and H > 0:
                    g1_hybrid()
            while stage_tiles:
                g1_consume()
            if G1H_AT >= len(stages) and H > 0:
                g1_hybrid()
            assert g1_done == NT

            # rotation: rotT = coefT*ca + swap_pairs(coefT)*sa (sa sign-folded)
            CSW = small.tile([P, B], f32, tag="csw")
            mask = [i ^ 1 for i in range(32)]
            nc.vector.stream_shuffle(CSW[:], CO[:], mask)
            TMP = small.tile([PC, B], f32, tag="tmp")
            TMP2 = small.tile([PC, B], f32, tag="tmp2")
            ROTT = small.tile([PC, B], f16, tag="rott")
            nc.vector.tensor_mul(TMP[:], CO[0:PC, :], CASA[:, 0:B])
            nc.vector.tensor_mul(TMP2[:], CSW[0:PC, :], CASA[:, B : 2 * B])
            nc.vector.tensor_add(ROTT[:], TMP[:], TMP2[:])

            # GEMM2: outT_j = bc_j^T @ rotT, 64 j-tiles per PSUM bank.
            # Bank evacs alternate ACT/DVE; paired output DMAs ride both
            # HWDGE rings, ordered so no DMA blocks a later evac on its ring.
            OST = big.tile([P, NT * B], f16, tag="ost")
            ngroups = (NT + GSZ - 1) // GSZ
            bank_w = []
            for g in range(ngroups):
                js = range(g * GSZ, min((g + 1) * GSZ, NT))
                PO = pop.tile([P, GSZ * B], f32, tag="po")
                for i, j in enumerate(js):
                    nc.tensor.matmul(
                        PO[:, i * B : (i + 1) * B],
                        BC[:, j * P : (j + 1) * P],
                        ROTT[:],
                        start=True,
                        stop=True,
                    )
                w = len(js) * B
                o0 = g * GSZ * B
                if g % 2 == 0:
                    nc.scalar.copy(OST[:, o0 : o0 + w], PO[:, 0:w])
                else:
                    nc.vector.tensor_copy(OST[:, o0 : o0 + w], PO[:, 0:w])
                bank_w.append(w)
                if g == 1:
                    nc.sync.dma_start(
                        out[:, 0 : bank_w[0] + bank_w[1]],
                        OST[:, 0 : bank_w[0] + bank_w[1]],
                    )
            o0 = bank_w[0] + bank_w[1]
            w = sum(bank_w[2:])
            nc.scalar.dma_start(out[:, o0 : o0 + w], OST[:, o0 : o0 + w])
    _split_excess_waits(nc, mybir)
    return nc


_CACHE = {}


def _get_nc():
    if "nc" not in _CACHE:
        _CACHE["nc"] = _build()
    return _CACHE["nc"]


def prep_in_maps(x, basis, angles):
    x = np.asarray(x)
    basis = np.asarray(basis)
    angles = np.asarray(angles).astype(np.float32)

    X2 = x.reshape(B, IJ)
    BF = basis.reshape(NL2, IJ)
    xt16 = np.ascontiguousarray(
        X2.T.reshape(NT, P, B).transpose(1, 0, 2)
    ).reshape(P, NT * B).astype(np.float16)

    j = np.arange(PC)
    sign = np.where(j % 2 == 0, 1.0, -1.0).astype(np.float32)

    in_maps = []
    for k in range(NCORES):
        bc16 = BF[k * PC : (k + 1) * PC].astype(np.float16)     # natural order
        hw = max(H, 1) * PC
        bijh = np.zeros((P, hw + 32), dtype=np.float16)
        if H > 0:
            bijh[:, 0:hw] = np.ascontiguousarray(
                bc16[:, TR * P :].reshape(PC, H, P).transpose(2, 1, 0)
            ).reshape(P, H * PC)
        lvals = ((k * 50 + j // 2) % 20).astype(np.float32)
        theta = lvals[:, None] * angles[None, :]                # [100, 8]
        casa = np.concatenate(
            [np.cos(theta), np.sin(theta) * sign[:, None]], axis=1
        ).astype(np.float32)                                    # [100, 16]
        bijh[0:PC, hw : hw + 32] = casa.view(np.float16)
        in_maps.append({"bc": bc16, "xt": xt16, "bijh": bijh})
    return in_maps


def kernel(x, basis, angles):
    from concourse.bass_utils import run_bass_kernel_spmd

    in_maps = prep_in_maps(x, basis, angles)
    res = run_bass_kernel_spmd(_get_nc(), in_maps, list(range(NCORES)))
    stage = np.zeros((P, NT * B), dtype=np.float32)
    for k in range(NCORES):
        stage += res.results[k]["out"].astype(np.float32)
    # col 8*j + b, row p  ->  out[b, 128*j + p]
    total = stage.reshape(P, NT, B).transpose(2, 1, 0).reshape(B, IJ)
    return np.ascontiguousarray(total).reshape(B, 1, 160, 160)


# revision 34
# speedup vs baseline: 1.0742x; 1.0240x over previous
"""Harmonic decomposition kernel for 8 TRN2 NeuronCores.

out[b] = basis^T R(theta_b) (basis @ x_b)   with per-harmonic complex rotation.

Sharding: the N*L*2 = 800 coefficient axis is split into 8 slices of 100 in
natural order (real parts on even rows, imaginary on odd rows). Each core
ships its basis slice ONCE in c-major layout (bc [100, 25600] f16), derives
the ij-major tiles needed by the projection on-chip (PE transpose -> PSUM f16
-> DVE/ACT evacuation), and runs both GEMMs with the basis slice as the
*stationary* matmul operand so each matmul only streams 8 batch columns:

  GEMM1 (projection):     coefT[c, b] += bijT_t[ij, c]^T @ xt_t[ij, b]
  rotate:                 rotT = coefT*ca + shuffle(coefT)*sa  (partition-pair
                          swap via stream_shuffle mask i^1; sa sign-folded)
  GEMM2 (reconstruction): outT_j[ij, b] = bc_j[c, ij]^T @ rotT[c, b]

A tunable tail of H ij-tiles is shipped pre-transposed from the host (bijh)
to balance PE-transpose cycles against DMA bandwidth and shorten the tail.
GEMM2 results are DMA'd straight from PSUM as f32 partials (no staging hop);
the host sums the 8 partials.
"""

import sys

import numpy as np

for _p in ("/opt/trn_rl_repo",):
    if _p not in sys.path:
        sys.path.insert(0, _p)

B = 8          # batch
IJ = 25600     # 160*160 spatial
NL2 = 800     # total coefficients
PC = 100       # coefficients per core
P = 128        # partitions
NT = 200       # ij tiles (IJ / 128)
NCORES = 8

# --- tunables ---------------------------------------------------------------
H = 44                                  # hybrid tiles shipped pre-transposed
TR = NT - H                             # tiles transposed on-chip
CH_TR = [8] + [20] * 6 + [16, 8, 4]     # bc chunk sizes over t < TR (sum 156)
CH_G2 = [26, 10, 8]                     # bc chunk sizes over t >= TR
STAGES = [8] + [10] * 13 + [6, 8, 4]    # transpose stages (chunk-aligned)
G1_LAG = 2                              # stages between transpose and G1 use
G1H_AT = 15                             # stage index after which G1-H is issued
XT_SPLIT = 40                           # tiles in the early xt slice
XT_A_AFTER = 2                          # chunk index after which xt_a ships
XT_B_AFTER = 4                          # chunk index after which xt_b ships
NSTG = 5                                # SBUF stage buffers
NPT = 3                                 # PSUM transpose buffers
GSZ = 64                                # GEMM2 j-tiles per PSUM bank
NPO = 4                                 # GEMM2 PSUM banks
assert sum(CH_TR) == TR and sum(CH_G2) == H and sum(STAGES) == TR


def _patch_tile_drain():
    """This container's walrus caps sem-waits at 1 per instruction; the stock
    Tile tail drain carries one wait per live semaphore. Keep one on the drain
    and emit the rest as individual SP wait instructions before the barrier."""
    import concourse.tile as tile
    from concourse.vector_clock import ScopedClock

    if getattr(tile.TileContext, "_ant_drain_patched", False):
        return

    def _drain_and_barrier(self, tick_clock, wait_clock):
        nc = self.nc
        drain_inst = nc.sync.drain()
        wait_clock.add_sem_waits(
            drain_inst.ins, ScopedClock({None: tick_clock.global_clock})
        )
        si = drain_inst.ins.sync_info
        waits = list(si.on_wait) if si and si.on_wait else []
        if len(waits) > 1:
            num2sem = {s.num: s for s in self.sems.allocated().values()}
            si.on_wait = waits[:1]
            for w in waits[1:]:
                op = {"sem-ge-imm": "sem-ge", "sem-eq-imm": "sem-eq"}[w.wait_mode]
                nc.sync.nop(nofuse=True).wait_op(num2sem[w.id], w.wait_value, op)
        nc.all_engine_barrier()
        assert self.sems is not None
        popped = nc._tile_sem_poison_stack.pop()
        assert popped is self._sem_poison
        nc.clear_and_free_semaphores(list(self.sems.allocated().values()))
        nc.all_engine_barrier()

    tile.TileContext._drain_and_barrier = _drain_and_barrier
    tile.TileContext._ant_drain_patched = True


def _split_excess_waits(nc, mybir):
    """Walrus in this container accepts at most 1 sem-wait per instruction
    (2 for EventSemaphore). Tile can attach several. Move the extras onto
    fresh NoOps inserted just before the instruction on the same engine —
    same-engine streams execute in order, so semantics are preserved."""
    ctr = 0
    for fn in nc.m.functions:
        for bb in fn.blocks:
            out, changed = [], False
            for inst in bb.instructions:
                si = inst.sync_info
                waits = list(si.on_wait) if si and si.on_wait else []
                cap = 2 if isinstance(inst, mybir.InstEventSemaphore) else 1
                if len(waits) > cap:
                    for w in waits[:-cap]:
                        nop = mybir.InstNoOp(name=f"I-wsplit-{ctr}", ins=[], outs=[])
                        ctr += 1
                        nop.engine = inst.engine
                        nop.sync_info = mybir.SyncInfo(on_wait=[w], on_update=[])
                        out.append(nop)
                    si.on_wait = waits[-cap:]
                    changed = True
                out.append(inst)
            if changed:
                bb.instructions = out


def _build():
    import concourse.bass as bass
    import concourse.mybir as mybir
    import concourse.tile as tile
    from concourse.masks import make_identity

    _patch_tile_drain()
    f16 = mybir.dt.float16
    f32 = mybir.dt.float32

    nc = bass.Bass()
    bc = nc.declare_dram_parameter("bc", [PC, IJ], f16, isOutput=False)
    xt = nc.declare_dram_parameter("xt", [P, NT * B], f16, isOutput=False)
    # bijh carries the hybrid pre-transposed tiles plus 32 trailing f16
    # columns holding the f32 rotation table (ca|sa) bitcast to f16 words
    bijh = nc.declare_dram_parameter(
        "bijh", [P, max(H, 1) * PC + 32], f16, isOutput=False
    )
    # outT tiles packed [ij_local 128, tile, b]: col 8*j + b = out_b[128*j + p]
    out = nc.declare_dram_parameter("out", [P, NT * B], f16, isOutput=True)

    with tile.TileContext(nc) as tc:
        with (
            tc.tile_pool(name="big", bufs=1) as big,
            tc.tile_pool(name="small", bufs=1) as small,
            tc.tile_pool(name="stg", bufs=NSTG) as stg,
            tc.tile_pool(name="pt", bufs=NPT, space="PSUM") as ptp,
            tc.tile_pool(name="co", bufs=1, space="PSUM") as cop,
            tc.tile_pool(name="po", bufs=NPO, space="PSUM") as pop,
        ):
            BC = big.tile([PC, IJ], f16, tag="bc")
            XT = big.tile([P, NT * B], f16, tag="xt")
            BIJH = big.tile([P, max(H, 1) * PC + 32], f16, tag="bijh")
            CASA = BIJH[0:PC, max(H, 1) * PC : max(H, 1) * PC + 32].bitcast(f32)
            IDENT = small.tile([PC, PC], f16, tag="ident")

            # identity for PE transpose, built on the otherwise-idle gpsimd
            make_identity(nc, IDENT[:])

            # Input DMAs, FIFO on SP's ring: bc chunks with an early xt slice
            # (G1 consumes xt tile-by-tile) and the rest of xt a few chunks
            # later; bijh (which carries the rotation table) then the G2-only
            # bc range last, so the final arrival gates only GEMM2's smallest
            # bank.
            t0 = 0
            for i, cn in enumerate(CH_TR):
                nc.sync.dma_start(
                    BC[:, t0 * P : (t0 + cn) * P], bc[:, t0 * P : (t0 + cn) * P]
                )
                t0 += cn
                if i + 1 == XT_A_AFTER:
                    nc.sync.dma_start(
                        XT[:, 0 : XT_SPLIT * B], xt[:, 0 : XT_SPLIT * B]
                    )
                if i + 1 == XT_B_AFTER:
                    nc.sync.dma_start(
                        XT[:, XT_SPLIT * B :], xt[:, XT_SPLIT * B :]
                    )
            if H > 0:
                nc.sync.dma_start(BIJH[:], bijh[:])
            for cn in CH_G2:
                nc.sync.dma_start(BC[:, t0 * P : (t0 + cn) * P], bc[:, t0 * P : (t0 + cn) * P])
                t0 += cn

            # coefT accumulator [128, 8] f32; zero the pad quadrant before the
            # G1 group opens (start=True re-resets rows 0:100, leaving 100:128
            # zero for the partition shuffle below)
            CO = cop.tile([P, B], f32, tag="co")
            nc.vector.memset(CO[96:P, :], 0.0)

            # GEMM1 pipeline: PE transposes stage s into PSUM (f16), DVE/ACT
            # evacuate to an SBUF stage buffer, PE consumes the stage as
            # *stationary* weights (8-col matmuls) G1_LAG stages behind the
            # transposes so the PE->evac->PE round trip stays off the PE's
            # critical path.
            stages = []
            t0 = 0
            for n in STAGES:
                stages.append((t0, n))
                t0 += n
            stage_tiles = []   # (STG tile, stage) ready for G1
            g1_done = 0

            def g1_consume():
                nonlocal g1_done
                ST, (ct, cn) = stage_tiles.pop(0)
                for k in range(cn):
                    t = ct + k
                    nc.tensor.matmul(
                        CO[0:PC, :],
                        ST[:, k * PC : (k + 1) * PC],
                        XT[:, t * B : (t + 1) * B],
                        start=(g1_done == 0),
                        stop=(g1_done == NT - 1),
                    )
                    g1_done += 1

            def g1_hybrid():
                # hybrid pre-transposed tiles, issued mid-stream (once bijh has
                # landed) so they stay off the tail critical path
                nonlocal g1_done
                for i in range(H):
                    t = TR + i
                    nc.tensor.matmul(
                        CO[0:PC, :],
                        BIJH[:, i * PC : (i + 1) * PC],
                        XT[:, t * B : (t + 1) * B],
                        start=(g1_done == 0),
                        stop=(g1_done == NT - 1),
                    )
                    g1_done += 1

            for s, (ct, cn) in enumerate(stages):
                PT = ptp.tile([P, 10 * PC], f16, tag="pt")
                for k in range(cn):
                    nc.tensor.transpose(
                        PT[:, k * PC : (k + 1) * PC],
                        BC[:, (ct + k) * P : (ct + k + 1) * P],
                        IDENT[:],
                    )
                ST = stg.tile([P, 10 * PC], f16, tag="stg")
                if s % 2 == 1:
                    nc.scalar.copy(ST[:, 0 : cn * PC], PT[:, 0 : cn * PC])
                else:
                    nc.vector.tensor_copy(ST[:, 0 : cn * PC], PT[:, 0 : cn * PC])
                stage_tiles.append((ST, (ct, cn)))
                if len(stage_tiles) > G1_LAG:
                    g1_consume()
                if s == G1H_AT and H > 0:
                    g1_hybrid()
            while stage_tiles:
                g1_consume()
            if G1H_AT >= len(stages) and H > 0:
                g1_hybrid()
            assert g1_done == NT

            # rotation: rotT = coefT*ca + swap_pairs(coefT)*sa (sa sign-folded)
            CSW = small.tile([P, B], f32, tag="csw")
            mask = [i ^ 1 for i in range(32)]
            nc.vector.stream_shuffle(CSW[:], CO[:], mask)
            TMP = small.tile([PC, B], f32, tag="tmp")
            TMP2 = small.tile([PC, B], f32, tag="tmp2")
            ROTT = small.tile([PC, B], f16, tag="rott")
            nc.vector.tensor_mul(TMP[:], CO[0:PC, :], CASA[:, 0:B])
            nc.vector.tensor_mul(TMP2[:], CSW[0:PC, :], CASA[:, B : 2 * B])
            nc.vector.tensor_add(ROTT[:], TMP[:], TMP2[:])

            # GEMM2: outT_j = bc_j^T @ rotT, 64 j-tiles per PSUM bank.
            # Bank evacs alternate ACT/DVE; paired output DMAs ride both
            # HWDGE rings, ordered so no DMA blocks a later evac on its ring.
            OST = big.tile([P, NT * B], f16, tag="ost")
            ngroups = (NT + GSZ - 1) // GSZ
            bank_w = []
            for g in range(ngroups):
                js = range(g * GSZ, min((g + 1) * GSZ, NT))
                PO = pop.tile([P, GSZ * B], f32, tag="po")
                for i, j in enumerate(js):
                    nc.tensor.matmul(
                        PO[:, i * B : (i + 1) * B],
                        BC[:, j * P : (j + 1) * P],
                        ROTT[:],
                        start=True,
                        stop=True,
                    )
                w = len(js) * B
                o0 = g * GSZ * B
                if g % 2 == 0:
                    nc.scalar.copy(OST[:, o0 : o0 + w], PO[:, 0:w])
                else:
                    nc.vector.tensor_copy(OST[:, o0 : o0 + w], PO[:, 0:w])
                bank_w.append(w)
                if g == 1:
                    nc.sync.dma_start(
                        out[:, 0 : bank_w[0] + bank_w[1]],
                        OST[:, 0 : bank_w[0] + bank_w[1]],
                    )
            o0 = bank_w[0] + bank_w[1]
            w = sum(bank_w[2:])
            nc.scalar.dma_start(out[:, o0 : o0 + w], OST[:, o0 : o0 + w])
    _split_excess_waits(nc, mybir)
    return nc


_CACHE = {}


def _get_nc():
    if "nc" not in _CACHE:
        _CACHE["nc"] = _build()
    return _CACHE["nc"]


def prep_in_maps(x, basis, angles):
    x = np.asarray(x)
    basis = np.asarray(basis)
    angles = np.asarray(angles).astype(np.float32)

    X2 = x.reshape(B, IJ)
    BF = basis.reshape(NL2, IJ)
    xt16 = np.ascontiguousarray(
        X2.T.reshape(NT, P, B).transpose(1, 0, 2)
    ).reshape(P, NT * B).astype(np.float16)

    j = np.arange(PC)
    sign = np.where(j % 2 == 0, 1.0, -1.0).astype(np.float32)

    in_maps = []
    for k in range(NCORES):
        bc16 = BF[k * PC : (k + 1) * PC].astype(np.float16)     # natural order
        hw = max(H, 1) * PC
        bijh = np.zeros((P, hw + 32), dtype=np.float16)
        if H > 0:
            bijh[:, 0:hw] = np.ascontiguousarray(
                bc16[:, TR * P :].reshape(PC, H, P).transpose(2, 1, 0)
            ).reshape(P, H * PC)
        lvals = ((k * 50 + j // 2) % 20).astype(np.float32)
        theta = lvals[:, None] * angles[None, :]                # [100, 8]
        casa = np.concatenate(
            [np.cos(theta), np.sin(theta) * sign[:, None]], axis=1
        ).astype(np.float32)                                    # [100, 16]
        bijh[0:PC, hw : hw + 32] = casa.view(np.float16)
        in_maps.append({"bc": bc16, "xt": xt16, "bijh": bijh})
    return in_maps


def kernel(x, basis, angles):
    from concourse.bass_utils import run_bass_kernel_spmd

    in_maps = prep_in_maps(x, basis, angles)
    res = run_bass_kernel_spmd(_get_nc(), in_maps, list(range(NCORES)))
    stage = np.zeros((P, NT * B), dtype=np.float32)
    for k in range(NCORES):
        stage += res.results[k]["out"].astype(np.float32)
    # col 8*j + b, row p  ->  out[b, 128*j + p]
    total = stage.reshape(P, NT, B).transpose(2, 1, 0).reshape(B, IJ)
    return np.ascontiguousarray(total).reshape(B, 1, 160, 160)
